# revision 77
# baseline (speedup 1.0000x reference)
"""Trainium2 Bass kernel for ClassicAttention (B=2, S=2048, D=1024, H=16).

Sharding: tensor-parallel over heads across 8 cores (2 heads/core).
  - Host stages x^T (d-major, bf16) and bf16 per-core weight slices, so the
    kernel starts matmuls immediately (no on-device transpose/cast/gather).
  - QKV: each core computes Q^T, K^T (d-major) for its 2 heads plus
    row-major V, over all B*S rows.
  - Attention: transposed-scores S^T[k,q] so the exp output is directly
    P^T (the AV matmul's moving operand); softmax denominators ride a
    ones-column appended to V (row 64 of the AV accumulator). No
    max-subtraction (scores bounded here). Causal trim at 128-column
    granularity on both the scores and AV matmuls.
  - c_proj: per-half AllGather of per-core context (d-major); each core
    computes a 128-column slice of the output, transposed ([j, B*S]);
    the host transposes back. c_proj(b0) is interleaved into
    attention(b1) emission so the PE never waits on a collective.
All matmuls bf16 inputs with fp32 PSUM accumulation.
"""

import numpy as np
import ml_dtypes

import concourse.bass as bass
import concourse.tile as tile
import concourse.mybir as mybir
from concourse import bacc
from concourse.bass_utils import run_bass_kernel_spmd

F32 = mybir.dt.float32
BF16 = mybir.dt.bfloat16

NCORES = 8
B, S, D = 2, 2048, 1024
H, HD = 16, 64
HPC = H // NCORES          # heads per core = 2
M = B * S                  # 4096 rows
NSUP = M // 512            # 8 row-supers of 512
ST_B = S // 128            # 16 s-tiles per batch
KCH = D // 128             # 8 contraction chunks
G_PER_B = S // 512         # 4 q-supers per batch
SCALE = 1.0 / (HD ** 0.5)
EXP = mybir.ActivationFunctionType.Exp


def build_ir(nc):
    # ---------------- DRAM I/O ----------------
    xT = nc.dram_tensor("xT", [D, M], BF16, kind="ExternalInput").ap()
    wqk = nc.dram_tensor("wqk", [D, 256], BF16, kind="ExternalInput").ap()
    wv = nc.dram_tensor("wv", [D, 128], BF16, kind="ExternalInput").ap()
    wp = nc.dram_tensor("wp", [D, 128], BF16, kind="ExternalInput").ap()
    bqk = nc.dram_tensor("bqk", [256], F32, kind="ExternalInput").ap()
    bv = nc.dram_tensor("bv", [128], F32, kind="ExternalInput").ap()
    bp = nc.dram_tensor("bp", [128], F32, kind="ExternalInput").ap()
    outT = nc.dram_tensor("outT", [128, M], F32, kind="ExternalOutput").ap()

    # causal mask bias: cols 384-511 are the diagonal 128x128 triangle
    # (0 where q >= k, -240 → -30 after the 1/8 softmax scale), cols 0-383
    # are all -240 so a wider slice also blankets the fully-masked gap
    # between two causal-trimmed halves (enables one merged exp per tile).
    maskb_np = np.where(np.arange(128)[None, :] >= np.arange(128)[:, None],
                        0.0, -240.0)
    maskx_np = np.concatenate([np.full((128, 384), -240.0), maskb_np], axis=1)
    maskb_const = nc.inline_tensor(maskx_np.astype(ml_dtypes.bfloat16),
                                   "maskb_const").ap()
    ident_const = nc.inline_tensor(np.eye(128).astype(ml_dtypes.bfloat16),
                                   "ident_const").ap()

    rg = [list(range(NCORES))]

    with tile.TileContext(nc) as tc:
        _emit(nc, tc, xT, wqk, wv, wp, bqk, bv, bp, outT,
              (maskb_const, ident_const), rg)
    return nc


def _emit(nc, tc, xT, wqk, wv, wp, bqk, bv, bp, outT, consts, rg):
    maskb_const, ident_const = consts
    import contextlib
    es = contextlib.ExitStack()
    with es:
        singles = es.enter_context(tc.tile_pool(name="singles", bufs=1))
        dram = es.enter_context(tc.tile_pool(name="dram", bufs=1, space="DRAM"))

        # ------------- persistent SBUF -------------
        qT = singles.tile([128, M], BF16)          # [2 heads x 64 d, B*S]
        kT = singles.tile([128, M], BF16)
        v_sb = singles.tile([128, B * ST_B, 130], BF16)  # [Va|1|Vb|1] per s-tile
        maskb_sb = singles.tile([128, 512], BF16)
        ident_sb = singles.tile([128, 128], BF16)
        wqk_sb = singles.tile([128, KCH, 256], BF16)
        wv_sb = singles.tile([128, KCH, 128], BF16)
        wp_sb = singles.tile([128, KCH, 128], BF16)
        bqk_sb = singles.tile([128, 2], F32)
        bp_sb = singles.tile([128, 1], F32)
        bv_bc = singles.tile([128, 128], F32)
        ones_t = singles.tile([128, 64], F32)
        nc.vector.memset(ones_t, 1.0)

        # ------------- DMAs, critical-path first -------------
        xt_pool = es.enter_context(tc.tile_pool(name="xt", bufs=4))
        xT_r = xT.rearrange("(c p) m -> p c m", p=128)
        wqk_r = wqk.rearrange("(c p) j -> p c j", p=128)
        # first QKV matmul is gated on just these two small DMAs
        nc.sync.dma_start(out=wqk_sb[:, 0:2, :], in_=wqk_r[:, 0:2, :])
        xts = [xt_pool.tile([128, KCH, 512], BF16, tag="xtile",
                            name=f"xt{su}") for su in range(NSUP)]
        nc.sync.dma_start(out=xts[0][:, 0:2, :], in_=xT_r[:, 0:2, 0:512])
        nc.sync.dma_start(out=wqk_sb[:, 2:8, :], in_=wqk_r[:, 2:8, :])
        nc.sync.dma_start(out=xts[0][:, 2:4, :], in_=xT_r[:, 2:4, 0:512])
        nc.sync.dma_start(out=wv_sb, in_=wv.rearrange("(c p) j -> p c j", p=128))
        nc.sync.dma_start(out=bqk_sb, in_=bqk.rearrange("(t p) -> p t", p=128))
        # bv broadcast to all 128 partitions via 0-partition-stride DMA read
        nc.sync.dma_start(
            out=bv_bc,
            in_=bass.AP(tensor=bv.tensor, offset=bv.offset, ap=[[0, 128], [1, 128]]))
        nc.sync.dma_start(out=maskb_sb, in_=maskb_const)
        nc.sync.dma_start(out=ident_sb, in_=ident_const)
        nc.sync.dma_start(out=xts[0][:, 4:8, :], in_=xT_r[:, 4:8, 0:512])
        for su in range(1, NSUP):
            nc.sync.dma_start(out=xts[su],
                              in_=xT_r[:, :, su * 512:(su + 1) * 512])
        nc.sync.dma_start(out=wp_sb, in_=wp.rearrange("(c p) j -> p c j", p=128))
        nc.sync.dma_start(out=bp_sb, in_=bp.rearrange("(a p) -> p a", p=128))
        nc.vector.memset(v_sb, 1.0)                # ones columns pre-set

        # ------------- QKV projection -------------
        # aux_ps [128,512] slots serve Q/K/V accumulation AND c_proj, so
        # batch-1 QKV can interleave under batch-0 attention (8-bank budget:
        # scores 4 + aux 2 + ctx 2).
        aux_ps = es.enter_context(tc.tile_pool(name="aux_ps", bufs=2,
                                               space="PSUM"))

        def qkv_super(su):
            xtile = xts[su]
            # Q^T and K^T for this row-super (d-major, both heads stacked)
            for jt, dst in ((0, qT), (1, kT)):
                ps = aux_ps.tile([128, 512], F32, tag="qk", name=f"qk{su}{jt}")
                for kc in range(KCH):
                    nc.tensor.matmul(
                        ps,
                        lhsT=wqk_sb[:, kc, jt * 128:(jt + 1) * 128],
                        rhs=xtile[:, kc, :],
                        start=(kc == 0), stop=(kc == KCH - 1),
                    )
                nc.vector.tensor_scalar_add(
                    dst[:, su * 512:(su + 1) * 512], ps, bqk_sb[:, jt:jt + 1])
            # V (row-major) for the 4 s-tiles of this super
            for mt in range(4):
                st = su * 4 + mt   # global s-tile index
                ps = aux_ps.tile([128, 512], F32, tag="qk", name=f"v{su}{mt}")
                for kc in range(KCH):
                    nc.tensor.matmul(
                        ps[:, 0:128],
                        lhsT=xtile[:, kc, mt * 128:(mt + 1) * 128],
                        rhs=wv_sb[:, kc, :],
                        start=(kc == 0), stop=(kc == KCH - 1),
                    )
                for hl in range(HPC):
                    nc.vector.tensor_add(
                        v_sb[:, st, hl * 65:hl * 65 + 64],
                        ps[:, hl * 64:(hl + 1) * 64],
                        bv_bc[:, hl * 64:(hl + 1) * 64],
                    )

        for su in range(4):
            qkv_super(su)

        # ------------- attention + c_proj (interleaved) -------------
        # AllGather groups: 2MB messages where hidden, 1MB for the final one
        GROUPS = {0: [(0, 1), (2, 3)], 1: [(3,), (2, 1), (0,)]}
        ctx_loc, ctx_all = {}, {}
        for b in range(B):
            for gs in GROUPS[b]:
                w = 512 * len(gs)
                ctx_loc[(b, gs)] = dram.tile(
                    [128, w], BF16, tag=f"ctxl{b}{gs[0]}", name=f"ctxl{b}{gs[0]}")
                ctx_all[(b, gs)] = dram.tile(
                    [NCORES * 128, w], BF16, addr_space="Shared",
                    tag=f"ctxa{b}{gs[0]}", name=f"ctxa{b}{gs[0]}")
        pt_pool = es.enter_context(tc.tile_pool(name="pt", bufs=6))
        craw_pool = es.enter_context(tc.tile_pool(name="craw", bufs=8))
        post = es.enter_context(tc.tile_pool(name="post", bufs=4))
        cs_pool = es.enter_context(tc.tile_pool(name="cs", bufs=4))
        cg_pool = es.enter_context(tc.tile_pool(name="cg", bufs=3))
        osb = es.enter_context(tc.tile_pool(name="osb", bufs=2))
        # scores 2x[128,1024] (4 banks) + ctx accumulators (2 banks);
        # aux_ps above holds the remaining 2.
        s_ps = es.enter_context(tc.tile_pool(name="s_ps", bufs=2, space="PSUM"))
        ctx_ps = es.enter_context(tc.tile_pool(name="ctx_ps", bufs=2, space="PSUM"))

        craws = {}

        def attn_g(b, g):
            """Scores + exp + AV for one q-super, AV pipelined two kps behind."""
            n_kt = 4 * g + 4
            cps = [ctx_ps.tile([65, 512], F32, tag="ctx", name=f"cps{b}{g}{hl}")
                   for hl in range(HPC)]
            q_sl = [qT[hl * 64:(hl + 1) * 64,
                       b * S + g * 512:b * S + (g + 1) * 512]
                    for hl in range(HPC)]

            def emit_av(kp, pts):
                for half in (0, 1):
                    kt = 2 * kp + half
                    qo = max(kt - 4 * g, 0) * 128
                    for hl in range(HPC):
                        nc.tensor.matmul(
                            cps[hl][:, qo:512],
                            lhsT=v_sb[:, b * ST_B + kt, hl * 65:hl * 65 + 65],
                            rhs=pts[hl][:, half * 512 + qo:(half + 1) * 512],
                            start=(kt == 0), stop=(kt == n_kt - 1),
                        )

            pending = []
            for kp in range(n_kt // 2):
                sps = [s_ps.tile([128, 1024], F32, tag="s", name=f"sps{hl}")
                       for hl in range(HPC)]
                pts = [pt_pool.tile([128, 1024], BF16, tag="pt", name=f"pt{hl}")
                       for hl in range(HPC)]
                # scores: alternate heads so the two K=64 matmuls row-tile;
                # causal mask folded in as a -240 bias via an extra matmul
                for half in (0, 1):
                    kt = 2 * kp + half
                    qo = max(kt - 4 * g, 0) * 128  # causal trim offset
                    diag = kt - 4 * g >= 0
                    for hl in range(HPC):
                        nc.tensor.matmul(
                            sps[hl][:, half * 512 + qo:(half + 1) * 512],
                            lhsT=kT[hl * 64:(hl + 1) * 64,
                                    b * S + kt * 128:b * S + (kt + 1) * 128],
                            rhs=q_sl[hl][:, qo:512],
                            start=True, stop=not diag,
                            tile_position=(64 * hl, 0),
                            skip_group_check=True,
                        )
                        if diag:
                            # half 1's mask matmul widens to also OVERWRITE
                            # the cleared-but-unwritten gap [512, 512+qo) so
                            # one exp can span the whole tail of the tile
                            ext = qo if half == 1 else 0
                            lo = half * 512 + qo
                            nc.tensor.matmul(
                                sps[hl][:, lo - ext:lo + 128],
                                lhsT=ident_sb,
                                rhs=maskb_sb[:, 384 - ext:512],
                                start=False, stop=True,
                                skip_group_check=True,
                            )
                # exp
                for hl in range(HPC):
                    pt, sp = pts[hl], sps[hl]
                    if 2 * kp + 1 < 4 * g:        # both halves full
                        nc.scalar.activation(pt, sp, EXP, scale=SCALE)
                    else:                          # diagonal pair: one exp
                        qo_lo = max(2 * kp - 4 * g, 0) * 128
                        nc.scalar.activation(
                            pt[:, qo_lo:], sp[:, qo_lo:], EXP, scale=SCALE)
                pending.append((kp, pts))
                if len(pending) > 1:
                    emit_av(*pending.pop(0))
            for item in pending:
                emit_av(*item)

            for hl in range(HPC):
                craw = craw_pool.tile([65, 512], F32, tag="craw",
                                      name=f"craw{b}{g}{hl}")
                nc.vector.tensor_copy(craw, cps[hl])
                craws[(b, g, hl)] = craw

        def normalize_gs(b, gs):
            """Normalize ctx by softmax sums; DMA out; AllGather.

            The sums row (craw partition 64) is broadcast to 64 partitions
            with a K=1 PE outer product, then fast-reciprocal (~18 correct
            bits) and multiply — no DRAM round-trips on the trigger path."""
            n = len(gs)
            for hl in range(HPC):
                cs = cs_pool.tile([64, n, 512], BF16, tag=f"cs{n}")
                for i, g in enumerate(gs):
                    bc = aux_ps.tile([128, 512], F32, tag="qk",
                                     name=f"bc{b}{g}{hl}")
                    nc.tensor.matmul(
                        bc[0:64, :], lhsT=ones_t[64:65, :],
                        rhs=craws[(b, g, hl)][64:65, :],
                        start=True, stop=True, tile_position=(64, 0),
                    )
                    rc = post.tile([64, 512], F32, tag="rc")
                    nc.vector.reciprocal_approx_fast(out=rc, in_=bc[0:64, :])
                    nc.vector.tensor_mul(
                        cs[:, i, :], craws[(b, g, hl)][0:64, :], rc)
                nc.sync.dma_start(
                    out=ctx_loc[(b, gs)][hl * 64:(hl + 1) * 64, :], in_=cs)
            nc.gpsimd.collective_compute(
                "AllGather", mybir.AluOpType.bypass, replica_groups=rg,
                ins=[ctx_loc[(b, gs)].opt()], outs=[ctx_all[(b, gs)].opt()],
            )

        def cproj_load(b, gs, gate=None):
            cg = cg_pool.tile([128, KCH, 512 * len(gs)], BF16,
                              tag=f"cg{len(gs)}", bufs=2)
            if gate is not None:
                # 2-byte dummy write (overwritten by the real load) makes the
                # load — and the c_proj matmuls behind it — depend on late
                # attention progress, so the scheduler cannot slot them into
                # model-idle PE windows where they'd stall on the real
                # (launch-skew-delayed) AllGather.
                nc.sync.dma_start(out=cg[0:1, 0:1, 0:1], in_=gate[0:1, 0:1])
            nc.sync.dma_start(
                out=cg,
                in_=ctx_all[(b, gs)].rearrange("(c p) m -> p c m", p=128))
            return cg

        def cproj_gs(b, gs, cg):
            for i, g in enumerate(gs):
                ps = aux_ps.tile([128, 512], F32, tag="qk", name=f"cp{b}{g}")
                for c in range(NCORES):
                    nc.tensor.matmul(
                        ps, lhsT=wp_sb[:, c, :],
                        rhs=cg[:, c, i * 512:(i + 1) * 512],
                        start=(c == 0), stop=(c == NCORES - 1),
                    )
                o = osb.tile([128, 512], F32, tag="o")
                nc.vector.tensor_scalar_add(o, ps, bp_sb)
                nc.sync.dma_start(
                    out=outT[:, b * S + g * 512:b * S + (g + 1) * 512],
                    in_=o)

        # batch 0 attention starts right after the batch-0 QKV supers;
        # batch-1 QKV is emitted after (lower priority) and fills the PE
        # slack of the ACT-bound batch-0 attention.
        attn_g(0, 0)
        attn_g(0, 1)
        normalize_gs(0, (0, 1))
        attn_g(0, 2)
        attn_g(0, 3)
        normalize_gs(0, (2, 3))
        for su in range(4, NSUP):
            qkv_super(su)
        # batch 1 attention largest-q-super first (its AllGather triggers
        # earliest and overlaps the rest), batch-0 c_proj interleaved
        # Demote c_proj priority so the list scheduler never hoists its
        # matmuls into the in-order PE queue ahead of ready attention work
        # (hoisted c_proj stalls on the launch-skew-delayed AllGathers and
        # blocks the whole attention pipeline behind it).
        @contextlib.contextmanager
        def low_priority(off=1000000):
            p = tc.cur_priority
            tc.cur_priority = p + off
            try:
                yield
            finally:
                tc.cur_priority = p

        cgs = {}
        attn_g(1, 3)
        normalize_gs(1, (3,))
        attn_g(1, 2)
        attn_g(1, 1)
        normalize_gs(1, (2, 1))
        gate_mid = ctx_loc[(1, (2, 1))]
        cgs[(0, (0, 1))] = cproj_load(0, (0, 1), gate=gate_mid)
        cgs[(0, (2, 3))] = cproj_load(0, (2, 3), gate=gate_mid)
        with low_priority():
            cproj_gs(0, (0, 1), cgs[(0, (0, 1))])
            cproj_gs(0, (2, 3), cgs[(0, (2, 3))])
        attn_g(1, 0)
        normalize_gs(1, (0,))
        gate_end = ctx_loc[(1, (0,))]
        cgs[(1, (3,))] = cproj_load(1, (3,), gate=gate_end)
        with low_priority():
            cproj_gs(1, (3,), cgs[(1, (3,))])
        cgs[(1, (2, 1))] = cproj_load(1, (2, 1), gate=gate_end)
        with low_priority():
            cproj_gs(1, (2, 1), cgs[(1, (2, 1))])
        cgs[(1, (0,))] = cproj_load(1, (0,))
        with low_priority():
            cproj_gs(1, (0,), cgs[(1, (0,))])


_CACHE = {}


def _get_compiled():
    if "nc" not in _CACHE:
        nc = bacc.Bacc("TRN2", target_bir_lowering=False, debug=False,
                       num_devices=NCORES)
        build_ir(nc)
        nc.compile()
        _CACHE["nc"] = nc
    return _CACHE["nc"]


def make_in_maps(inputs):
    x = np.asarray(inputs["hidden_states"], dtype=np.float32)   # [B,S,D]
    wa = np.asarray(inputs["c_attn_w"], dtype=np.float32)       # [D, 3D]
    ba = np.asarray(inputs["c_attn_b"], dtype=np.float32)       # [3D]
    wpr = np.asarray(inputs["c_proj_w"], dtype=np.float32)      # [D, D]
    bpr = np.asarray(inputs["c_proj_b"], dtype=np.float32)      # [D]

    xT_bf = np.ascontiguousarray(
        x.reshape(M, D).T).astype(ml_dtypes.bfloat16)           # [D, M]
    wq, wk, wv_full = wa[:, 0:D], wa[:, D:2 * D], wa[:, 2 * D:3 * D]
    bq, bk, bv_full = ba[0:D], ba[D:2 * D], ba[2 * D:3 * D]

    in_maps = []
    for r in range(NCORES):
        hs = slice(r * HPC * HD, (r + 1) * HPC * HD)   # this core's head dims
        in_maps.append({
            "xT": xT_bf,
            "wqk": np.ascontiguousarray(np.concatenate(
                [wq[:, hs], wk[:, hs]], axis=1)).astype(ml_dtypes.bfloat16),
            "wv": np.ascontiguousarray(
                wv_full[:, hs]).astype(ml_dtypes.bfloat16),
            "wp": np.ascontiguousarray(
                wpr[:, r * 128:(r + 1) * 128]).astype(ml_dtypes.bfloat16),
            "bqk": np.ascontiguousarray(np.concatenate([bq[hs], bk[hs]])),
            "bv": np.ascontiguousarray(bv_full[hs]),
            "bp": np.ascontiguousarray(bpr[r * 128:(r + 1) * 128]),
        })
    return in_maps


def assemble(results):
    slices = [results[r]["outT"].T.reshape(B, S, 128) for r in range(NCORES)]
    return np.ascontiguousarray(np.concatenate(slices, axis=2).astype(np.float32))


def kernel(**inputs):
    in_maps = make_in_maps(inputs)
    nc = _get_compiled()
    res = run_bass_kernel_spmd(nc, in_maps, core_ids=list(range(NCORES)))
    return assemble(res.results)


if __name__ == "__main__":
    import reference
    inp = reference.setup_inputs()
    out = kernel(**{k: np.asarray(v) for k, v in inp.items()})
    print(out.shape, out.dtype)


# revision 78
# speedup vs baseline: 1.1571x; 1.1571x over previous
"""Trainium2 Bass kernel for ClassicAttention (B=2, S=2048, D=1024, H=16).

Sharding: tensor-parallel over heads across 8 cores (2 heads/core).
  - Host stages x^T (d-major, bf16) and bf16 per-core weight slices, so the
    kernel starts matmuls immediately (no on-device transpose/cast/gather).
  - QKV: each core computes Q^T, K^T (d-major) for its 2 heads plus
    row-major V, over all B*S rows.
  - Attention: transposed-scores S^T[k,q] so the exp output is directly
    P^T (the AV matmul's moving operand); softmax denominators ride a
    ones-column appended to V (row 64 of the AV accumulator). No
    max-subtraction (scores bounded here). Causal trim at 128-column
    granularity on both the scores and AV matmuls.
  - c_proj: per-half AllGather of per-core context (d-major); each core
    computes a 128-column slice of the output, transposed ([j, B*S]);
    the host transposes back. c_proj(b0) is interleaved into
    attention(b1) emission so the PE never waits on a collective.
All matmuls bf16 inputs with fp32 PSUM accumulation.
"""

import numpy as np
import ml_dtypes

import concourse.bass as bass
import concourse.tile as tile
import concourse.mybir as mybir
from concourse import bacc
from concourse.bass_utils import run_bass_kernel_spmd

F32 = mybir.dt.float32
BF16 = mybir.dt.bfloat16

NCORES = 8
B, S, D = 2, 2048, 1024
H, HD = 16, 64
HPC = H // NCORES          # heads per core = 2
M = B * S                  # 4096 rows
NSUP = M // 512            # 8 row-supers of 512
ST_B = S // 128            # 16 s-tiles per batch
KCH = D // 128             # 8 contraction chunks
G_PER_B = S // 512         # 4 q-supers per batch
SCALE = 1.0 / (HD ** 0.5)
EXP = mybir.ActivationFunctionType.Exp


def build_ir(nc):
    # ---------------- DRAM I/O ----------------
    xT = nc.dram_tensor("xT", [D, M], BF16, kind="ExternalInput").ap()
    wqk = nc.dram_tensor("wqk", [D, 256], BF16, kind="ExternalInput").ap()
    wv = nc.dram_tensor("wv", [D, 128], BF16, kind="ExternalInput").ap()
    wp = nc.dram_tensor("wp", [D, 128], BF16, kind="ExternalInput").ap()
    bqk = nc.dram_tensor("bqk", [256], F32, kind="ExternalInput").ap()
    bv = nc.dram_tensor("bv", [128], F32, kind="ExternalInput").ap()
    bp = nc.dram_tensor("bp", [128], F32, kind="ExternalInput").ap()
    outT = nc.dram_tensor("outT", [128, M], F32, kind="ExternalOutput").ap()

    # causal mask bias: cols 384-511 are the diagonal 128x128 triangle
    # (0 where q >= k, -240 → -30 after the 1/8 softmax scale), cols 0-383
    # are all -240 so a wider slice also blankets the fully-masked gap
    # between two causal-trimmed halves (enables one merged exp per tile).
    maskb_np = np.where(np.arange(128)[None, :] >= np.arange(128)[:, None],
                        0.0, -240.0)
    maskx_np = np.concatenate([np.full((128, 384), -240.0), maskb_np], axis=1)
    maskb_const = nc.inline_tensor(maskx_np.astype(ml_dtypes.bfloat16),
                                   "maskb_const").ap()
    ident_const = nc.inline_tensor(np.eye(128).astype(ml_dtypes.bfloat16),
                                   "ident_const").ap()

    rg = [list(range(NCORES))]

    with tile.TileContext(nc) as tc:
        _emit(nc, tc, xT, wqk, wv, wp, bqk, bv, bp, outT,
              (maskb_const, ident_const), rg)
    return nc


def _emit(nc, tc, xT, wqk, wv, wp, bqk, bv, bp, outT, consts, rg):
    maskb_const, ident_const = consts
    import contextlib
    es = contextlib.ExitStack()
    with es:
        singles = es.enter_context(tc.tile_pool(name="singles", bufs=1))
        dram = es.enter_context(tc.tile_pool(name="dram", bufs=1, space="DRAM"))

        # ------------- persistent SBUF -------------
        qT = singles.tile([128, M], BF16)          # [2 heads x 64 d, B*S]
        kT = singles.tile([128, M], BF16)
        v_sb = singles.tile([128, B * ST_B, 130], BF16)  # [Va|1|Vb|1] per s-tile
        maskb_sb = singles.tile([128, 512], BF16)
        ident_sb = singles.tile([128, 128], BF16)
        wqk_sb = singles.tile([128, KCH, 256], BF16)
        wv_sb = singles.tile([128, KCH, 128], BF16)
        wp_sb = singles.tile([128, KCH, 128], BF16)
        bqk_sb = singles.tile([128, 2], F32)
        bp_sb = singles.tile([128, 1], F32)
        bv_bc = singles.tile([128, 128], F32)

        # ------------- DMAs, critical-path first -------------
        xt_pool = es.enter_context(tc.tile_pool(name="xt", bufs=4))
        xT_r = xT.rearrange("(c p) m -> p c m", p=128)
        wqk_r = wqk.rearrange("(c p) j -> p c j", p=128)
        # first QKV matmul is gated on just these two small DMAs
        nc.sync.dma_start(out=wqk_sb[:, 0:2, :], in_=wqk_r[:, 0:2, :])
        xts = [xt_pool.tile([128, KCH, 512], BF16, tag="xtile",
                            name=f"xt{su}") for su in range(NSUP)]
        nc.sync.dma_start(out=xts[0][:, 0:2, :], in_=xT_r[:, 0:2, 0:512])
        nc.sync.dma_start(out=wqk_sb[:, 2:8, :], in_=wqk_r[:, 2:8, :])
        nc.sync.dma_start(out=xts[0][:, 2:4, :], in_=xT_r[:, 2:4, 0:512])
        nc.sync.dma_start(out=wv_sb, in_=wv.rearrange("(c p) j -> p c j", p=128))
        nc.sync.dma_start(out=bqk_sb, in_=bqk.rearrange("(t p) -> p t", p=128))
        # bv broadcast to all 128 partitions via 0-partition-stride DMA read
        nc.sync.dma_start(
            out=bv_bc,
            in_=bass.AP(tensor=bv.tensor, offset=bv.offset, ap=[[0, 128], [1, 128]]))
        nc.sync.dma_start(out=maskb_sb, in_=maskb_const)
        nc.sync.dma_start(out=ident_sb, in_=ident_const)
        nc.sync.dma_start(out=xts[0][:, 4:8, :], in_=xT_r[:, 4:8, 0:512])
        for su in range(1, NSUP):
            nc.sync.dma_start(out=xts[su],
                              in_=xT_r[:, :, su * 512:(su + 1) * 512])
        nc.sync.dma_start(out=wp_sb, in_=wp.rearrange("(c p) j -> p c j", p=128))
        nc.sync.dma_start(out=bp_sb, in_=bp.rearrange("(a p) -> p a", p=128))
        nc.vector.memset(v_sb, 1.0)                # ones columns pre-set

        # ------------- QKV projection -------------
        # aux_ps [128,512] slots serve Q/K/V accumulation AND c_proj, so
        # batch-1 QKV can interleave under batch-0 attention (8-bank budget:
        # scores 4 + aux 2 + ctx 2).
        aux_ps = es.enter_context(tc.tile_pool(name="aux_ps", bufs=2,
                                               space="PSUM"))

        def qkv_super(su):
            xtile = xts[su]
            # Q^T and K^T for this row-super (d-major, both heads stacked)
            for jt, dst in ((0, qT), (1, kT)):
                ps = aux_ps.tile([128, 512], F32, tag="qk", name=f"qk{su}{jt}")
                for kc in range(KCH):
                    nc.tensor.matmul(
                        ps,
                        lhsT=wqk_sb[:, kc, jt * 128:(jt + 1) * 128],
                        rhs=xtile[:, kc, :],
                        start=(kc == 0), stop=(kc == KCH - 1),
                    )
                nc.vector.tensor_scalar_add(
                    dst[:, su * 512:(su + 1) * 512], ps, bqk_sb[:, jt:jt + 1])
            # V (row-major) for the 4 s-tiles of this super
            for mt in range(4):
                st = su * 4 + mt   # global s-tile index
                ps = aux_ps.tile([128, 512], F32, tag="qk", name=f"v{su}{mt}")
                for kc in range(KCH):
                    nc.tensor.matmul(
                        ps[:, 0:128],
                        lhsT=xtile[:, kc, mt * 128:(mt + 1) * 128],
                        rhs=wv_sb[:, kc, :],
                        start=(kc == 0), stop=(kc == KCH - 1),
                    )
                for hl in range(HPC):
                    nc.vector.tensor_add(
                        v_sb[:, st, hl * 65:hl * 65 + 64],
                        ps[:, hl * 64:(hl + 1) * 64],
                        bv_bc[:, hl * 64:(hl + 1) * 64],
                    )

        for su in range(4):
            qkv_super(su)

        # ------------- attention + c_proj (interleaved) -------------
        # AllGather groups: 2MB messages where hidden, 1MB for the final one
        GROUPS = {0: [(0, 1), (2, 3)], 1: [(3,), (2, 1), (0,)]}
        ctx_loc, ctx_all = {}, {}
        for b in range(B):
            for gs in GROUPS[b]:
                w = 512 * len(gs)
                ctx_loc[(b, gs)] = dram.tile(
                    [128, w], BF16, tag=f"ctxl{b}{gs[0]}", name=f"ctxl{b}{gs[0]}")
                ctx_all[(b, gs)] = dram.tile(
                    [NCORES * 128, w], BF16, addr_space="Shared",
                    tag=f"ctxa{b}{gs[0]}", name=f"ctxa{b}{gs[0]}")
        pt_pool = es.enter_context(tc.tile_pool(name="pt", bufs=6))
        craw_pool = es.enter_context(tc.tile_pool(name="craw", bufs=8))
        post = es.enter_context(tc.tile_pool(name="post", bufs=4))
        cs_pool = es.enter_context(tc.tile_pool(name="cs", bufs=4))
        cg_pool = es.enter_context(tc.tile_pool(name="cg", bufs=3))
        osb = es.enter_context(tc.tile_pool(name="osb", bufs=2))
        # scores 2x[128,1024] (4 banks) + ctx accumulators (2 banks);
        # aux_ps above holds the remaining 2.
        s_ps = es.enter_context(tc.tile_pool(name="s_ps", bufs=2, space="PSUM"))
        ctx_ps = es.enter_context(tc.tile_pool(name="ctx_ps", bufs=2, space="PSUM"))

        craws = {}

        def attn_g(b, g):
            """Scores + exp + AV for one q-super, AV pipelined two kps behind."""
            n_kt = 4 * g + 4
            cps = [ctx_ps.tile([65, 512], F32, tag="ctx", name=f"cps{b}{g}{hl}")
                   for hl in range(HPC)]
            q_sl = [qT[hl * 64:(hl + 1) * 64,
                       b * S + g * 512:b * S + (g + 1) * 512]
                    for hl in range(HPC)]

            def emit_av(kp, pts):
                for half in (0, 1):
                    kt = 2 * kp + half
                    qo = max(kt - 4 * g, 0) * 128
                    for hl in range(HPC):
                        nc.tensor.matmul(
                            cps[hl][:, qo:512],
                            lhsT=v_sb[:, b * ST_B + kt, hl * 65:hl * 65 + 65],
                            rhs=pts[hl][:, half * 512 + qo:(half + 1) * 512],
                            start=(kt == 0), stop=(kt == n_kt - 1),
                        )

            pending = []
            for kp in range(n_kt // 2):
                sps = [s_ps.tile([128, 1024], F32, tag="s", name=f"sps{hl}")
                       for hl in range(HPC)]
                pts = [pt_pool.tile([128, 1024], BF16, tag="pt", name=f"pt{hl}")
                       for hl in range(HPC)]
                # scores: alternate heads so the two K=64 matmuls row-tile;
                # causal mask folded in as a -240 bias via an extra matmul
                for half in (0, 1):
                    kt = 2 * kp + half
                    qo = max(kt - 4 * g, 0) * 128  # causal trim offset
                    diag = kt - 4 * g >= 0
                    for hl in range(HPC):
                        nc.tensor.matmul(
                            sps[hl][:, half * 512 + qo:(half + 1) * 512],
                            lhsT=kT[hl * 64:(hl + 1) * 64,
                                    b * S + kt * 128:b * S + (kt + 1) * 128],
                            rhs=q_sl[hl][:, qo:512],
                            start=True, stop=not diag,
                            tile_position=(64 * hl, 0),
                            skip_group_check=True,
                        )
                        if diag:
                            # half 1's mask matmul widens to also OVERWRITE
                            # the cleared-but-unwritten gap [512, 512+qo) so
                            # one exp can span the whole tail of the tile
                            ext = qo if half == 1 else 0
                            lo = half * 512 + qo
                            nc.tensor.matmul(
                                sps[hl][:, lo - ext:lo + 128],
                                lhsT=ident_sb,
                                rhs=maskb_sb[:, 384 - ext:512],
                                start=False, stop=True,
                                skip_group_check=True,
                            )
                # exp
                for hl in range(HPC):
                    pt, sp = pts[hl], sps[hl]
                    if 2 * kp + 1 < 4 * g:        # both halves full
                        nc.scalar.activation(pt, sp, EXP, scale=SCALE)
                    else:                          # diagonal pair: one exp
                        qo_lo = max(2 * kp - 4 * g, 0) * 128
                        nc.scalar.activation(
                            pt[:, qo_lo:], sp[:, qo_lo:], EXP, scale=SCALE)
                pending.append((kp, pts))
                if len(pending) > 1:
                    emit_av(*pending.pop(0))
            for item in pending:
                emit_av(*item)

            for hl in range(HPC):
                craw = craw_pool.tile([65, 512], F32, tag="craw",
                                      name=f"craw{b}{g}{hl}")
                nc.vector.tensor_copy(craw, cps[hl])
                craws[(b, g, hl)] = craw

        def normalize_gs(b, gs):
            """Reciprocal of sums for a group of q-supers; scale; AllGather."""
            n = len(gs)
            sums_dr = dram.tile([2 * n, 512], F32, tag=f"sums_dr{n}", bufs=2,
                                name=f"sums_dr{b}{gs[0]}")
            for hl in range(HPC):
                for i, g in enumerate(gs):
                    nc.sync.dma_start(
                        out=sums_dr[hl * n + i:hl * n + i + 1, :],
                        in_=craws[(b, g, hl)][64:65, :])
            # repack [2n,512] -> [64,16n] so reciprocal uses 64 partitions
            sums_sb = post.tile([64, 16 * n], F32, tag=f"sums{n}")
            sums_src = bass.AP(tensor=sums_dr.tensor, offset=sums_dr.offset,
                               ap=[[16 * n, 64], [1, 16 * n]])
            nc.sync.dma_start(out=sums_sb, in_=sums_src)
            recip_sb = post.tile([64, 16 * n], F32, tag=f"recip{n}")
            nc.vector.reciprocal(recip_sb, sums_sb)
            recip_dr = dram.tile([2 * n, 512], F32, tag=f"recip_dr{n}", bufs=2,
                                 name=f"recip_dr{b}{gs[0]}")
            recip_dst = bass.AP(tensor=recip_dr.tensor, offset=recip_dr.offset,
                                ap=[[16 * n, 64], [1, 16 * n]])
            nc.sync.dma_start(out=recip_dst, in_=recip_sb)
            bc_sb = post.tile([64, 2 * n, 512], F32, tag=f"bc{n}")
            bc_src = bass.AP(tensor=recip_dr.tensor, offset=recip_dr.offset,
                             ap=[[0, 64]] + list(recip_dr.ap))
            nc.sync.dma_start(out=bc_sb, in_=bc_src)
            for hl in range(HPC):
                cs = cs_pool.tile([64, n, 512], BF16, tag=f"cs{n}")
                for i, g in enumerate(gs):
                    nc.vector.tensor_mul(
                        cs[:, i, :], craws[(b, g, hl)][0:64, :],
                        bc_sb[:, hl * n + i, :])
                nc.sync.dma_start(
                    out=ctx_loc[(b, gs)][hl * 64:(hl + 1) * 64, :], in_=cs)
            nc.gpsimd.collective_compute(
                "AllGather", mybir.AluOpType.bypass, replica_groups=rg,
                ins=[ctx_loc[(b, gs)].opt()], outs=[ctx_all[(b, gs)].opt()],
            )

        def cproj_load(b, gs, gate=None):
            cg = cg_pool.tile([128, KCH, 512 * len(gs)], BF16,
                              tag=f"cg{len(gs)}", bufs=2)
            if gate is not None:
                # 2-byte dummy write (overwritten by the real load) makes the
                # load — and the c_proj matmuls behind it — depend on late
                # attention progress, so the scheduler cannot slot them into
                # model-idle PE windows where they'd stall on the real
                # (launch-skew-delayed) AllGather.
                nc.sync.dma_start(out=cg[0:1, 0:1, 0:1], in_=gate[0:1, 0:1])
            nc.sync.dma_start(
                out=cg,
                in_=ctx_all[(b, gs)].rearrange("(c p) m -> p c m", p=128))
            return cg

        def cproj_gs(b, gs, cg):
            for i, g in enumerate(gs):
                ps = aux_ps.tile([128, 512], F32, tag="qk", name=f"cp{b}{g}")
                for c in range(NCORES):
                    nc.tensor.matmul(
                        ps, lhsT=wp_sb[:, c, :],
                        rhs=cg[:, c, i * 512:(i + 1) * 512],
                        start=(c == 0), stop=(c == NCORES - 1),
                    )
                o = osb.tile([128, 512], F32, tag="o")
                nc.vector.tensor_scalar_add(o, ps, bp_sb)
                nc.sync.dma_start(
                    out=outT[:, b * S + g * 512:b * S + (g + 1) * 512],
                    in_=o)

        # batch 0 attention starts right after the batch-0 QKV supers;
        # batch-1 QKV is emitted after (lower priority) and fills the PE
        # slack of the ACT-bound batch-0 attention.
        attn_g(0, 0)
        attn_g(0, 1)
        normalize_gs(0, (0, 1))
        attn_g(0, 2)
        attn_g(0, 3)
        normalize_gs(0, (2, 3))
        for su in range(4, NSUP):
            qkv_super(su)
        # batch 1 attention largest-q-super first (its AllGather triggers
        # earliest and overlaps the rest), batch-0 c_proj interleaved
        # Demote c_proj priority so the list scheduler never hoists its
        # matmuls into the in-order PE queue ahead of ready attention work
        # (hoisted c_proj stalls on the launch-skew-delayed AllGathers and
        # blocks the whole attention pipeline behind it).
        @contextlib.contextmanager
        def low_priority(off=1000000):
            p = tc.cur_priority
            tc.cur_priority = p + off
            try:
                yield
            finally:
                tc.cur_priority = p

        cgs = {}
        attn_g(1, 3)
        normalize_gs(1, (3,))
        attn_g(1, 2)
        attn_g(1, 1)
        normalize_gs(1, (2, 1))
        gate_mid = ctx_loc[(1, (2, 1))]
        cgs[(0, (0, 1))] = cproj_load(0, (0, 1), gate=gate_mid)
        cgs[(0, (2, 3))] = cproj_load(0, (2, 3), gate=gate_mid)
        with low_priority():
            cproj_gs(0, (0, 1), cgs[(0, (0, 1))])
            cproj_gs(0, (2, 3), cgs[(0, (2, 3))])
        attn_g(1, 0)
        normalize_gs(1, (0,))
        gate_end = ctx_loc[(1, (0,))]
        cgs[(1, (3,))] = cproj_load(1, (3,), gate=gate_end)
        with low_priority():
            cproj_gs(1, (3,), cgs[(1, (3,))])
        cgs[(1, (2, 1))] = cproj_load(1, (2, 1), gate=gate_end)
        with low_priority():
            cproj_gs(1, (2, 1), cgs[(1, (2, 1))])
        cgs[(1, (0,))] = cproj_load(1, (0,))
        with low_priority():
            cproj_gs(1, (0,), cgs[(1, (0,))])


_CACHE = {}


def _get_compiled():
    if "nc" not in _CACHE:
        nc = bacc.Bacc("TRN2", target_bir_lowering=False, debug=False,
                       num_devices=NCORES)
        build_ir(nc)
        nc.compile()
        _CACHE["nc"] = nc
    return _CACHE["nc"]


def make_in_maps(inputs):
    x = np.asarray(inputs["hidden_states"], dtype=np.float32)   # [B,S,D]
    wa = np.asarray(inputs["c_attn_w"], dtype=np.float32)       # [D, 3D]
    ba = np.asarray(inputs["c_attn_b"], dtype=np.float32)       # [3D]
    wpr = np.asarray(inputs["c_proj_w"], dtype=np.float32)      # [D, D]
    bpr = np.asarray(inputs["c_proj_b"], dtype=np.float32)      # [D]

    xT_bf = np.ascontiguousarray(
        x.reshape(M, D).T).astype(ml_dtypes.bfloat16)           # [D, M]
    wq, wk, wv_full = wa[:, 0:D], wa[:, D:2 * D], wa[:, 2 * D:3 * D]
    bq, bk, bv_full = ba[0:D], ba[D:2 * D], ba[2 * D:3 * D]

    in_maps = []
    for r in range(NCORES):
        hs = slice(r * HPC * HD, (r + 1) * HPC * HD)   # this core's head dims
        in_maps.append({
            "xT": xT_bf,
            "wqk": np.ascontiguousarray(np.concatenate(
                [wq[:, hs], wk[:, hs]], axis=1)).astype(ml_dtypes.bfloat16),
            "wv": np.ascontiguousarray(
                wv_full[:, hs]).astype(ml_dtypes.bfloat16),
            "wp": np.ascontiguousarray(
                wpr[:, r * 128:(r + 1) * 128]).astype(ml_dtypes.bfloat16),
            "bqk": np.ascontiguousarray(np.concatenate([bq[hs], bk[hs]])),
            "bv": np.ascontiguousarray(bv_full[hs]),
            "bp": np.ascontiguousarray(bpr[r * 128:(r + 1) * 128]),
        })
    return in_maps


def assemble(results):
    slices = [results[r]["outT"].T.reshape(B, S, 128) for r in range(NCORES)]
    return np.ascontiguousarray(np.concatenate(slices, axis=2).astype(np.float32))


def kernel(**inputs):
    in_maps = make_in_maps(inputs)
    nc = _get_compiled()
    res = run_bass_kernel_spmd(nc, in_maps, core_ids=list(range(NCORES)))
    return assemble(res.results)


if __name__ == "__main__":
    import reference
    inp = reference.setup_inputs()
    out = kernel(**{k: np.asarray(v) for k, v in inp.items()})
    print(out.shape, out.dtype)
